# revision 8
# baseline (speedup 1.0000x reference)
"""Banded multi-headed attention (nn_BandedMultiheadedAttention) on 8 Trainium2 NeuronCores.

Sharding: data-parallel over (batch, sequence-chunk): core c handles batch c//4,
query positions [256*(c%4), 256*(c%4)+256). Band halo (max (KC-1)*dil = 248) is
loaded per-core with zero padding (projection of zero rows reproduces the
reference's bias padding exactly).

Per-core pipeline (all layouts noted as [partition, free]):
  1. Q/K projections (fp32): qT_s/kT_s [dh=128, pos] from host-transposed inputs.
  2. Banded scores per subhead (fp32): scores_s [q, m] = qT_s^T @ kT_s window.
  3. Scores -> DRAM plane (deinterleaved by m%dil), shear-gather back as
     band [q, 32] (diagonal extraction becomes a strided DMA with contiguous
     32-element runs), PE-transpose to bandT [32, q].
  4. Pos_Sampling (Sk) matmul + Sb -> score2 [q, (head, 32)], exp (no max
     subtraction; max |score2| ~ 64 is safe in fp32), per-head row-sum,
     reciprocal, normalize -> W [q, (head, 32)] cast to bf16.
  5. W scatter -> zeroed DRAM plane per head in [q, m] layout (contiguous runs),
     xbar DMA-transpose readback -> W^T [m, q] bf16 tiles.
  6. V projection (bf16, heads packed 4-wide in N) -> v_h [m, dh] tiles.
  7. PV: attnT_h [dh, q] = v_h^T @ W^T (bf16, fp32 accum) + Vb (softmax rows
     sum to 1, so the value bias contributes exactly Vb).
  8. Collapse: out [q, 640] = sum_h attnT_h^T @ CkT_h + Cb (bf16 weights).
"""

import contextlib
import ctypes
import sys
import types

import numpy as np
import ml_dtypes

# ---------------------------------------------------------------- constants
B, N, D = 2, 1024, 640
DH, KC, SUBHEADS, HEADS = 128, 32, 5, 14
Q = 256                      # query positions per core
NCORES = 8
HALO = 124                   # (KC-1)*max_dil // 2
KV = 512                     # kv halo positions per core ([t0-124, t0+388))

DIL_S = [1, 1, 2, 4, 8]
SUPER = [5, 5, 2, 1, 1]
DIL_H = [1] * 10 + [2, 2, 4, 8]
S_OF_H = [0] * 5 + [1] * 5 + [2, 2, 3, 4]
PL_S = [(KC - 1) * d // 2 for d in DIL_S]          # [15,15,31,62,124]
OFF_S = [HALO - p for p in PL_S]                   # kT col of m=0 per subhead
PL_H = [(KC - 1) * d // 2 for d in DIL_H]
OFF_H = [HALO - p for p in PL_H]

M_S = [288, 288, 320, 384, 512]                    # scores plane width per subhead
SOFF = [0, 288, 576, 896, 1280]
SLD = 1792                                         # scores plane row stride

# W plane width per head: md = M_H/dil must be a multiple of 128 so that each
# 128-col chunk of the deinterleaved plane lies in a single residue class.
M_H = [384] * 10 + [512, 512, 512, 1024]
WOFF = [384 * h for h in range(10)] + [3840, 4352, 4864, 5376]
WLD = 6400                                         # W plane row stride
MC_H = [m // 128 for m in M_H]                     # m-chunks per head
KVX = 1024                                         # zero-extended vT columns

# V-projection head packs (same dilation within a pack)
PACKS = [[0, 1, 2, 3], [4, 5, 6, 7], [8, 9], [10, 11], [12], [13]]
PACK_OF_H = {h: (p, g.index(h)) for p, g in enumerate(PACKS) for h in g}
PACK_OFF = [OFF_H[g[0]] for g in PACKS]
PACK_MC = [MC_H[g[0]] for g in PACKS]

# head-column layout in the 448-wide score2/W tiles
HJ = HEADS * KC  # 448

_BUILT = None


def _inject_ntff_hook():
    """bass_utils reads antenv.axon_hooks for NTFF profiling; the module is
    absent in this image. Recreate the ctypes glue (mirrors trn_boot.py)."""
    try:
        import antenv.axon_hooks  # noqa: F401
        return
    except ImportError:
        pass

    def _make(so_path):
        try:
            lib = ctypes.CDLL(so_path)
        except OSError:
            return None
        if not hasattr(lib, "axon_start_nrt_profile"):
            return None
        lib.axon_start_nrt_profile.argtypes = [ctypes.POINTER(ctypes.c_int64), ctypes.c_size_t]
        lib.axon_start_nrt_profile.restype = ctypes.c_int64
        lib.axon_stop_nrt_profile.argtypes = [ctypes.c_char_p]
        lib.axon_stop_nrt_profile.restype = ctypes.c_int64

        @contextlib.contextmanager
        def _hook(output_dir, device_ids):
            import jax
            jax.devices()
            if device_ids:
                ids = (ctypes.c_int64 * len(device_ids))(*device_ids)
                rc = lib.axon_start_nrt_profile(ids, len(device_ids))
            else:
                rc = lib.axon_start_nrt_profile(None, 0)
            if rc != 0:
                raise RuntimeError(f"axon_start_nrt_profile rc={rc}")
            try:
                yield
            finally:
                n = lib.axon_stop_nrt_profile(str(output_dir).encode())
                print(f"ntff profile: {n} file(s) -> {output_dir}", file=sys.stderr)

        return _hook

    hook = _make("/opt/axon/libaxon_pjrt.so")
    mod = types.ModuleType("antenv.axon_hooks")
    mod.get_axon_ntff_profile_hook = lambda: hook
    mod.set_axon_ntff_profile_hook = lambda h: None
    sys.modules["antenv.axon_hooks"] = mod


def _build():
    """Build the (single) SPMD Bass program. Returns finalized nc."""
    import concourse.bass as bass
    import concourse.tile as tile
    from concourse import bacc, mybir
    from concourse.masks import make_identity

    f32 = mybir.dt.float32
    bf16 = mybir.dt.bfloat16
    AP = bass.AP

    nc = bacc.Bacc("TRN2", target_bir_lowering=False, debug=False, num_devices=NCORES)

    # ---------------- external IO
    qT_d = nc.dram_tensor("qT", [D, Q], f32, kind="ExternalInput")
    kT_d = nc.dram_tensor("kT", [D, KV], f32, kind="ExternalInput")
    vT_d = nc.dram_tensor("vT", [D, KV], bf16, kind="ExternalInput")
    QkT_d = nc.dram_tensor("QkT", [SUBHEADS, D, DH], f32, kind="ExternalInput")
    KkT_d = nc.dram_tensor("KkT", [SUBHEADS, D, DH], f32, kind="ExternalInput")
    Vp_d = [nc.dram_tensor(f"VkT{p}", [D, len(g) * DH], bf16, kind="ExternalInput")
            for p, g in enumerate(PACKS)]
    SkT_d = nc.dram_tensor("SkT", [KC, HJ], f32, kind="ExternalInput")
    Sb_d = nc.dram_tensor("Sb", [1, HJ], f32, kind="ExternalInput")
    QbT_d = nc.dram_tensor("QbT", [DH, SUBHEADS], f32, kind="ExternalInput")
    KbT_d = nc.dram_tensor("KbT", [DH, SUBHEADS], f32, kind="ExternalInput")
    VbT_d = nc.dram_tensor("VbT", [DH, HEADS], f32, kind="ExternalInput")
    CkT_d = nc.dram_tensor("CkT", [HEADS * DH, D], bf16, kind="ExternalInput")
    Cb_d = nc.dram_tensor("Cb", [1, D], f32, kind="ExternalInput")
    out_d = nc.dram_tensor("out", [Q, D], f32, kind="ExternalOutput")

    # ---------------- internal DRAM scratch (concrete offsets for shear APs)
    splane = nc.dram_tensor("splane", [Q, SLD], f32, kind="Internal")
    wplane = nc.dram_tensor("wplane", [Q, WLD], bf16, kind="Internal")

    with tile.TileContext(nc) as tc, contextlib.ExitStack() as ctx:
        consts = ctx.enter_context(tc.tile_pool(name="consts", bufs=1))
        acts = ctx.enter_context(tc.tile_pool(name="acts", bufs=1))
        work = ctx.enter_context(tc.tile_pool(name="work", bufs=4))
        wftp = ctx.enter_context(tc.tile_pool(name="wft", bufs=6))
        ps_mm = ctx.enter_context(tc.tile_pool(name="ps_mm", bufs=2, space="PSUM"))
        ps_sm = ctx.enter_context(tc.tile_pool(name="ps_sm", bufs=3, space="PSUM"))
        ps_at = ctx.enter_context(tc.tile_pool(name="ps_at", bufs=2, space="PSUM"))
        ps_co = ctx.enter_context(tc.tile_pool(name="ps_co", bufs=1, space="PSUM"))

        # ---------------- load inputs / weights
        qT = acts.tile([DH, SUBHEADS, Q], f32)          # query^T, dm-chunks on free
        nc.sync.dma_start(out=qT, in_=AP(qT_d, 0, [[Q, DH], [DH * Q, SUBHEADS], [1, Q]]))
        kT = acts.tile([DH, SUBHEADS, KV], f32)
        nc.sync.dma_start(out=kT, in_=AP(kT_d, 0, [[KV, DH], [DH * KV, SUBHEADS], [1, KV]]))
        vT = acts.tile([DH, SUBHEADS, KVX], bf16)
        nc.vector.memset(vT, 0.0)
        nc.sync.dma_start(
            out=AP(vT.tensor, vT.offset, [[SUBHEADS * KVX, DH], [KVX, SUBHEADS], [1, KV]]),
            in_=AP(vT_d, 0, [[KV, DH], [DH * KV, SUBHEADS], [1, KV]]))

        # [dm_par, s*5+dm_chunk, dh] (merged middle dim keeps the DMA AP 3-dim)
        QkT = consts.tile([DH, SUBHEADS * SUBHEADS, DH], f32)
        nc.sync.dma_start(
            out=QkT,
            in_=AP(QkT_d, 0, [[DH, DH], [DH * DH, SUBHEADS * SUBHEADS], [1, DH]]),
        )
        KkT = consts.tile([DH, SUBHEADS * SUBHEADS, DH], f32)
        nc.sync.dma_start(
            out=KkT,
            in_=AP(KkT_d, 0, [[DH, DH], [DH * DH, SUBHEADS * SUBHEADS], [1, DH]]),
        )
        Vp = []
        for p, g in enumerate(PACKS):
            npk = len(g) * DH
            t = consts.tile([DH, SUBHEADS, npk], bf16, name=f"Vp{p}")
            nc.sync.dma_start(out=t, in_=AP(Vp_d[p], 0, [[npk, DH], [DH * npk, SUBHEADS], [1, npk]]))
            Vp.append(t)
        SkT = consts.tile([KC, HJ], f32)
        nc.sync.dma_start(out=SkT, in_=SkT_d.ap())
        Sb = consts.tile([DH, HJ], f32)
        nc.sync.dma_start(out=Sb, in_=AP(Sb_d, 0, [[0, DH], [1, HJ]]))
        QbT = consts.tile([DH, SUBHEADS], f32)
        nc.sync.dma_start(out=QbT, in_=QbT_d.ap())
        KbT = consts.tile([DH, SUBHEADS], f32)
        nc.sync.dma_start(out=KbT, in_=KbT_d.ap())
        VbT = consts.tile([DH, HEADS], f32)
        nc.sync.dma_start(out=VbT, in_=VbT_d.ap())
        CkT = consts.tile([DH, HEADS, D], bf16)   # f-chunk h on partitions' free dim
        nc.sync.dma_start(out=CkT, in_=AP(CkT_d, 0, [[D, DH], [DH * D, HEADS], [1, D]]))
        Cb = consts.tile([DH, D], f32)
        nc.sync.dma_start(out=Cb, in_=AP(Cb_d, 0, [[0, DH], [1, D]]))

        ident = consts.tile([DH, DH], f32)
        make_identity(nc, ident)

        # ---------------- zero the W plane (two 128-row DMAs)
        zrow = consts.tile([DH, WLD], bf16)
        nc.vector.memset(zrow, 0.0)
        for c in range(2):
            nc.sync.dma_start(
                out=AP(wplane, c * 128 * WLD, [[WLD, 128], [1, WLD]]),
                in_=zrow,
            )

        # ---------------- Q/K projections (fp32)
        qTs, kTs = [], []
        for s in range(SUBHEADS):
            pq = ps_mm.tile([DH, Q], f32, name=f"pq{s}", tag="mm")
            for dc in range(SUBHEADS):
                nc.tensor.matmul(pq, QkT[:, s * SUBHEADS + dc, :], qT[:, dc, :],
                                 start=(dc == 0), stop=(dc == SUBHEADS - 1))
            t = acts.tile([DH, Q], f32, name=f"qTs{s}")
            nc.scalar.activation(t, pq, mybir.ActivationFunctionType.Identity,
                                 bias=QbT[:, s : s + 1], scale=1.0)
            qTs.append(t)

            pk = ps_mm.tile([DH, KV], f32, name=f"pk{s}", tag="mm")
            for dc in range(SUBHEADS):
                nc.tensor.matmul(pk, KkT[:, s * SUBHEADS + dc, :], kT[:, dc, :],
                                 start=(dc == 0), stop=(dc == SUBHEADS - 1))
            t = acts.tile([DH, KV], f32, name=f"kTs{s}")
            nc.scalar.activation(t, pk, mybir.ActivationFunctionType.Identity,
                                 bias=KbT[:, s : s + 1], scale=1.0)
            kTs.append(t)

        # ---------------- banded scores -> deinterleaved DRAM planes
        for s in range(SUBHEADS):
            dil, ms = DIL_S[s], M_S[s]
            for c in range(2):
                pscore = ps_mm.tile([128, ms], f32, name=f"psc{s}{c}", tag="mm")
                nc.tensor.matmul(pscore, qTs[s][:, c * 128 : c * 128 + 128],
                                 kTs[s][:, OFF_S[s] : OFF_S[s] + ms],
                                 start=True, stop=True)
                ssb = work.tile([128, ms], f32, name="ssb", tag="ssb", bufs=4)
                if dil == 1:
                    nc.scalar.copy(ssb, pscore)
                else:
                    # deinterleave m -> (m%dil, m//dil) during PSUM->SBUF copy
                    src = AP(pscore.tensor, pscore.offset,
                             [[ms, 128], [1, dil], [dil, ms // dil]])
                    dst = AP(ssb.tensor, ssb.offset,
                             [[ms, 128], [ms // dil, dil], [1, ms // dil]])
                    nc.scalar.copy(dst, src)
                nc.sync.dma_start(
                    out=AP(splane, c * 128 * SLD + SOFF[s], [[SLD, 128], [1, ms]]),
                    in_=ssb,
                )

        # ---------------- V projection (bf16, packed heads)
        vtiles = {}  # (pack, mc) -> [128, len(g)*128] bf16; rows in deint m-order
        for p, g in enumerate(PACKS):
            npk = len(g) * DH
            dil = DIL_H[g[0]]
            nb = M_H[g[0]] // (128 * dil)   # 128-col div-blocks per residue
            for mc in range(PACK_MC[p]):
                rho, db = mc // nb, mc % nb
                # chunk rows are m = off + dil*(db*128 + t) + rho, t in [0,128)
                cbase = PACK_OFF[p] + dil * db * 128 + rho
                pv = ps_mm.tile([128, npk], f32, name=f"pv{p}{mc}", tag="mm")
                for dc in range(SUBHEADS):
                    lhsT = AP(vT.tensor, vT.offset + dc * KVX + cbase,
                              [[SUBHEADS * KVX, DH], [dil, 128]])
                    nc.tensor.matmul(pv, lhsT, Vp[p][:, dc, :],
                                     start=(dc == 0), stop=(dc == SUBHEADS - 1))
                t = acts.tile([128, npk], bf16, name=f"v{p}_{mc}")
                nc.scalar.copy(t, pv)
                vtiles[(p, mc)] = t

        # ---------------- band extract + Sk + softmax -> W (bf16)
        Wq = []  # per qchunk: [128, 448] bf16
        for c in range(2):
            bandTs = []
            for s in range(SUBHEADS):
                dil, ms = DIL_S[s], M_S[s]
                band = work.tile([128, KC], f32, name="band", tag="band", bufs=6)
                base = c * 128 * SLD + SOFF[s]
                if dil == 1:
                    nc.sync.dma_start(out=band,
                                      in_=AP(splane, base + c * 128, [[SLD + 1, 128], [1, KC]]))
                else:
                    # SBUF APs address partitions only via dim 0: one DMA per
                    # residue rho, partition-strided by dil.
                    for rho in range(dil):
                        nc.sync.dma_start(
                            out=AP(band.tensor, band.offset + rho * KC,
                                   [[dil * KC, 128 // dil], [1, KC]]),
                            in_=AP(splane,
                                   base + (c * 128) // dil + rho * (SLD + ms // dil),
                                   [[dil * SLD + 1, 128 // dil], [1, KC]]),
                        )
                pbt = ps_sm.tile([KC, 128], f32, name="pbt", tag="sm")
                nc.tensor.transpose(pbt, band, ident)
                bt = work.tile([KC, 128], f32, name="bt", tag="bt", bufs=6)
                nc.scalar.copy(bt, pbt)
                bandTs.append(bt)

            e = work.tile([128, HJ], f32, name="e", tag="e", bufs=2)
            hlo = 0
            for s in range(SUBHEADS):
                ncols = SUPER[s] * KC
                psk = ps_sm.tile([128, ncols], f32, name="psk", tag="sm")
                nc.tensor.matmul(psk, bandTs[s], SkT[:, hlo : hlo + ncols],
                                 start=True, stop=True)
                # += Sb then exp
                nc.vector.tensor_add(e[:, hlo : hlo + ncols], psk,
                                     Sb[:, hlo : hlo + ncols])
                hlo += ncols
            nc.scalar.activation(e, e, mybir.ActivationFunctionType.Exp)
            z = work.tile([128, HEADS], f32, name="z", tag="z", bufs=4)
            nc.vector.reduce_sum(z, e.rearrange("p (h k) -> p h k", k=KC),
                                 axis=mybir.AxisListType.X)
            rz = work.tile([128, HEADS], f32, name="rz", tag="z", bufs=4)
            nc.vector.reciprocal(rz, z)
            w = work.tile([128, HJ], bf16, name="w", tag="w", bufs=2)
            nc.vector.tensor_mul(
                w.rearrange("p (h k) -> p h k", k=KC),
                e.rearrange("p (h k) -> p h k", k=KC),
                AP(rz.tensor, rz.offset, [[HEADS, 128], [1, HEADS], [0, KC]]),
            )
            Wq.append(w)

            # ---- scatter W into the zeroed plane ([q, m] layout, contiguous runs)
            # dil=1 heads 0..9 merged into one DMA
            nc.sync.dma_start(
                out=AP(wplane, c * 128 * (WLD + 1), [[WLD + 1, 128], [384, 10], [1, KC]]),
                in_=AP(w.tensor, w.offset, [[HJ, 128], [KC, 10], [1, KC]]),
            )
            for h in range(10, HEADS):
                dil, mh = DIL_H[h], M_H[h]
                base = c * 128 * WLD + WOFF[h] + (c * 128) // dil
                for rho in range(dil):
                    nc.sync.dma_start(
                        out=AP(wplane, base + rho * (WLD + mh // dil),
                               [[dil * WLD + 1, 128 // dil], [1, KC]]),
                        in_=AP(w.tensor, w.offset + h * KC + rho * HJ,
                               [[dil * HJ, 128 // dil], [1, KC]]),
                    )

        # ---------------- xbar readback + PV + collapse
        atiles = []
        for h in range(HEADS):
            p, hh = PACK_OF_H[h]
            pat = ps_at.tile([DH, Q], f32, name=f"pat{h}", tag="at")
            for mc in range(MC_H[h]):
                wft = wftp.tile([128, Q], bf16, name="wft", tag="wft")
                for c in range(2):
                    nc.sync.dma_start(
                        out=wft[:, c * 128 : c * 128 + 128],
                        in_=AP(wplane, (c * 128) * WLD + WOFF[h] + mc * 128,
                               [[WLD, 128], [1, 128]]),
                        transpose=True,
                    )
                nc.tensor.matmul(pat, vtiles[(p, mc)][:, hh * DH : hh * DH + DH], wft,
                                 start=(mc == 0), stop=(mc == MC_H[h] - 1))
            at = acts.tile([DH, Q], bf16, name=f"at{h}")
            nc.scalar.activation(at, pat, mybir.ActivationFunctionType.Identity,
                                 bias=VbT[:, h : h + 1], scale=1.0)
            atiles.append(at)

        outsb = [work.tile([128, D], f32, name=f"osb{c}", tag="osb", bufs=2) for c in range(2)]
        for c in range(2):
            for half in range(2):
                pc = ps_co.tile([128, 320], f32, name=f"pc{c}{half}", tag="co")
                for h in range(HEADS):
                    nc.tensor.matmul(pc, atiles[h][:, c * 128 : c * 128 + 128],
                                     CkT[:, h, half * 320 : half * 320 + 320],
                                     start=(h == 0), stop=(h == HEADS - 1))
                nc.vector.tensor_add(
                    outsb[c][:, half * 320 : half * 320 + 320], pc,
                    Cb[:, half * 320 : half * 320 + 320],
                )
            nc.sync.dma_start(
                out=AP(out_d, c * 128 * D, [[D, 128], [1, D]]),
                in_=outsb[c],
            )

    nc.finalize()
    return nc


def _prep_in_maps(inputs):
    bf = ml_dtypes.bfloat16
    query = np.asarray(inputs["query"], np.float32)
    key = np.asarray(inputs["key"], np.float32)
    value = np.asarray(inputs["value"], np.float32)
    Qk = np.asarray(inputs["Qk"], np.float32)
    Qb = np.asarray(inputs["Qb"], np.float32)
    Kk = np.asarray(inputs["Kk"], np.float32)
    Kb = np.asarray(inputs["Kb"], np.float32)
    Vk = np.asarray(inputs["Vk"], np.float32)
    Vb = np.asarray(inputs["Vb"], np.float32)
    Sk = np.asarray(inputs["Sk"], np.float32)
    Sb = np.asarray(inputs["Sb"], np.float32)
    Ck = np.asarray(inputs["Ck"], np.float32)
    Cb = np.asarray(inputs["Cb"], np.float32)

    QkT = np.ascontiguousarray(Qk.transpose(0, 2, 1))          # [5, 640, 128]
    KkT = np.ascontiguousarray(Kk.transpose(0, 2, 1))
    VkT = Vk.transpose(0, 2, 1)                                # [14, 640, 128]
    Vp = [np.ascontiguousarray(
            np.concatenate([VkT[h] for h in g], axis=1)).astype(bf)
          for g in PACKS]
    SkT = np.ascontiguousarray(Sk.transpose(2, 0, 1).reshape(KC, HJ))
    Sbr = np.ascontiguousarray(Sb.reshape(1, HJ))
    QbT = np.ascontiguousarray(Qb.T)                           # [128, 5]
    KbT = np.ascontiguousarray(Kb.T)
    VbT = np.ascontiguousarray(Vb.T)                           # [128, 14]
    CkT = np.ascontiguousarray(Ck.T).astype(bf)                # [1792, 640]
    Cbr = np.ascontiguousarray(Cb.reshape(1, D))

    in_maps = []
    for c in range(NCORES):
        b, t0 = c // 4, (c % 4) * Q
        kpad = np.zeros((KV, D), np.float32)
        vpad = np.zeros((KV, D), np.float32)
        lo, hi = max(0, t0 - HALO), min(N, t0 + Q + 132)
        kpad[lo - (t0 - HALO) : hi - (t0 - HALO)] = key[b, lo:hi]
        vpad[lo - (t0 - HALO) : hi - (t0 - HALO)] = value[b, lo:hi]
        m = {
            "qT": np.ascontiguousarray(query[b, t0 : t0 + Q].T),
            "kT": np.ascontiguousarray(kpad.T),
            "vT": np.ascontiguousarray(vpad.T).astype(bf),
            "QkT": QkT, "KkT": KkT,
            "SkT": SkT, "Sb": Sbr, "QbT": QbT, "KbT": KbT, "VbT": VbT,
            "CkT": CkT, "Cb": Cbr,
        }
        for p in range(len(PACKS)):
            m[f"VkT{p}"] = Vp[p]
        in_maps.append(m)
    return in_maps


def _run(inputs, trace=False, tmpdir=None):
    global _BUILT
    _inject_ntff_hook()
    from concourse.bass_utils import run_bass_kernel_spmd

    if _BUILT is None:
        _BUILT = _build()
    in_maps = _prep_in_maps(inputs)
    r = run_bass_kernel_spmd(_BUILT, in_maps, core_ids=list(range(NCORES)),
                             trace=trace, tmpdir=tmpdir)
    out = np.empty((B, N, D), np.float32)
    for c in range(NCORES):
        b, t0 = c // 4, (c % 4) * Q
        out[b, t0 : t0 + Q] = r.results[c]["out"]
    return out, r


def kernel(**inputs) -> np.ndarray:
    out, _ = _run(inputs, trace=False)
    return out
